# revision 60
# baseline (speedup 1.0000x reference)
"""TRN2 Bass/Tile kernel for nn_DotProductAttention (softmax over the QUERY axis).

reference:
    scores  = einsum('bqd,bkd->bqk', q, k) / sqrt(64)
    weights = softmax(scores, axis=1)          # over q, NOT k!
    out     = einsum('bqk,bkd->bqd', weights, v)

Because the softmax normalizes over q for each (b, k) column, we work with the
transposed score matrix T = K @ Q^T (shape [k, q]): the reduction axis (q) is
then the free axis, which the ACT accum_out reduction handles for free, and the
normalizer Z[k] lives on the contraction axis of the second matmul so it can be
folded into V (V' = V / Z) instead of rescaling the whole [k, q] tile.

Sharding: B=16 batches, data-parallel over 8 cores => 2 batches per core.
The two batches of a core are packed into the two 64-partition halves of
[128, *] tiles ((b, d) packing), which lets pairs of matmuls run concurrently
in disjoint PE-array row strips (scores) / column strips (AV).
"""

import math
from contextlib import ExitStack

import numpy as np

import concourse.bass as bass  # noqa: F401  (kept for symmetry with docs)
import concourse.mybir as mybir
import concourse.tile as tile
from bass_rust import add_dep_helper
from concourse import bacc, bass_utils
from concourse.masks import make_identity

FP32 = mybir.dt.float32
BF16 = mybir.dt.bfloat16

N_CORES = 8
B_FULL = 16
BPC = B_FULL // N_CORES  # batches per core = 2
S = 2048
D = 64
NCH = S // 128  # 16 key chunks of 128
SCALE = 1.0 / math.sqrt(D)


def emit_kernel(ctx: ExitStack, tc, q, k, v, o):
    """Emit the per-core Tile program. q/k/v/o are DRAM APs of [BPC, S, D] f32."""
    nc = tc.nc

    const_pool = ctx.enter_context(tc.tile_pool(name="const", bufs=1))
    big = ctx.enter_context(tc.tile_pool(name="big", bufs=1))
    dram = ctx.enter_context(tc.tile_pool(name="dram", bufs=1, space="DRAM"))
    # PSUM: phase B1 double-buffers two [128,2048] score tiles (all 8 banks);
    # phase B2 reuses the same pool for the [128,2048] O^T accumulator and
    # the [128,128] transpose tiles.
    ps = ctx.enter_context(tc.tile_pool(name="ps", bufs=2, space="PSUM"))

    ident = const_pool.tile([128, 128], FP32, name="ident")
    make_identity(nc, ident)
    identb = const_pool.tile([128, 128], BF16, name="identb")
    make_identity(nc, identb)
    zw = const_pool.tile([128, 128], BF16, name="zw")
    nc.vector.memset(zw[:], 0.0)

    # (b,d)-packed transposed operands: partitions 0:64 = batch0 d, 64:128 = batch1 d.
    QT = big.tile([128, S], BF16, name="QT")
    KT = big.tile([128, S], BF16, name="KT")
    # staging for Q/K chunks in (m, b, d) column layout, s on partitions
    qstage = big.tile([128, S], FP32, name="qstage")
    kstage = big.tile([128, S], FP32, name="kstage")
    # V chunks [128 k, 64 d] (f32 as loaded) and Vs = V / Z (bf16)
    V = big.tile([128, BPC * NCH * D], FP32, name="V")
    Vs = big.tile([128, BPC * NCH * D], BF16, name="Vs")
    # per (b, chunk) stats columns: [z, 1/z]
    stats = big.tile([128, BPC * NCH * 2], FP32, name="stats")
    # E[(b*NCH+i)*S :+ S] = exp(scores/sqrt(D)): [128 k, 2048 q] bf16, fully resident
    E = big.tile([128, BPC * NCH * S], BF16, name="E")
    # O^T staging ((b,d) packed on partitions, q on free), f32
    OT = big.tile([128, S], FP32, name="OT")
    # O in natural layout: column chunk m holds [q-tile m, (b d)]
    O_all = big.tile([128, S], FP32, name="O_all")
    # dummy destination for the DVE-side normalizer reductions
    zscr = big.tile([128, S], BF16, name="zscr")

    # bf16 copies of the staged (s, (b d)) matrices
    qbf = big.tile([128, S], BF16, name="qbf")
    kbf = big.tile([128, S], BF16, name="kbf")
    kbf_dram = dram.tile([S, 128], BF16, name="kbf_dram")

    # ---------------- phase A: load + transpose Q/K, load V ----------------
    # Loads: quarter-granularity strided stage DMAs, batch 0 on the sync
    # HWDGE queue and batch 1 on the scalar queue; V on the otherwise-idle
    # SWDGE (gpsimd) path. Transposes: chunk 0's scores read ALL of QT plus
    # KT chunk 0, so q0..15+k0 go through fast PE transposes (bf16, psum)
    # with the copies on the pre-B1-idle ACT engine; KT chunks 1..15 arrive
    # slightly later via one whole-tensor xbar DMA (bf16 DRAM roundtrip).
    # All stage DMAs ride the sync queue: the scalar queue must stay clear for
    # the transpose copies + ACTIVATEs (a stalled DMA there blocks B1), and q
    # loads fully before k so chunk 0 can start earliest.
    QRT = NCH // 4  # chunks per quarter-DMA
    last_q_dma = None
    # k's first quarter is needed by the transpose chain before q's last two
    # quarters, so interleave it into the load stream
    for src, stg, Q in (
        (q, qstage, 0),
        (q, qstage, 1),
        (k, kstage, 0),
        (q, qstage, 2),
        (q, qstage, 3),
        (k, kstage, 1),
        (k, kstage, 2),
        (k, kstage, 3),
    ):
        ssl = slice(Q * QRT * 128, (Q + 1) * QRT * 128)
        for b in range(BPC):
            dma = nc.sync.dma_start(
                stg[:, ssl].rearrange("p (m b d) -> p m b d", m=QRT, b=BPC, d=D)[
                    :, :, b, :
                ],
                src[b, ssl, :].rearrange("(m p) d -> p m d", p=128),
            )
            if src is q:
                last_q_dma = dma
    for b in range(BPC):
        vdma = nc.gpsimd.dma_start(
            V[:].rearrange("p (b m d) -> p b m d", b=BPC, m=NCH)[:, b, :, :],
            v[b].rearrange("(m p) d -> p m d", p=128),
        )
        # V isn't needed until well into B1 — keep its HBM traffic out of the
        # way of the critical q loads
        add_dep_helper(
            vdma.ins, last_q_dma.ins, sync=True, reason="delay V behind q loads"
        )
    for Q in range(4):
        csl = slice(Q * QRT * 128, (Q + 1) * QRT * 128)
        nc.vector.tensor_copy(qbf[:, csl], qstage[:, csl])
        nc.vector.tensor_copy(kbf[:, csl], kstage[:, csl])
    # PE-transposed chunks: q0..15 then k0..3 (the chunks B1 needs before the
    # DRAM-roundtrip xbar below can deliver the rest of KT)
    for idx, (t, m) in enumerate(
        [("q", mm) for mm in range(8)]
        + [("k", mm) for mm in range(4)]
        + [("q", mm) for mm in range(8, NCH)]
    ):
        bft, dst = (qbf, QT) if t == "q" else (kbf, KT)
        pt = ps.tile([128, 128], BF16, tag="ps", name=f"pt_{t}{m}")
        nc.tensor.transpose(pt[:], bft[:, m * 128 : (m + 1) * 128], identb[:])
        # alternate the psum-drain copies between the two idle engines so the
        # copy stage isn't the chain's rate limiter
        if idx % 2 == 0:
            nc.scalar.copy(dst[:, m * 128 : (m + 1) * 128], pt[:])
        else:
            nc.vector.tensor_copy(dst[:, m * 128 : (m + 1) * 128], pt[:])
    # KT chunks 4..15 via DRAM-roundtrip whole-tensor xbar transpose
    nc.sync.dma_start(
        kbf_dram[512:S, :].rearrange("(m p) c -> p m c", p=128),
        kbf[:, 512:S].rearrange("p (m c) -> p m c", m=NCH - 4),
    )
    nc.sync.dma_start_transpose(out=KT[:, 512:S], in_=kbf_dram[512:S, :])

    # ---------------- phase B1: scores -> exp, double-buffered --------------
    # Two [128,2048] score tiles rotate through all 8 PSUM banks; one N=2048
    # exp per (batch, chunk) with accum_out giving the softmax normalizer Z
    # directly (softmax axis == free axis).
    for i in range(NCH):
        for b in range(BPC):
            sct = ps.tile([128, S], FP32, tag="ps", name=f"sc{i}_{b}")
            for j in range(4):
                nc.tensor.matmul(
                    sct[:, j * 512 : (j + 1) * 512],
                    lhsT=KT[b * 64 : (b + 1) * 64, i * 128 : (i + 1) * 128],
                    rhs=QT[b * 64 : (b + 1) * 64, j * 512 : (j + 1) * 512],
                    start=True,
                    stop=True,
                )
            sb = (b * NCH + i) * 2
            eb = (b * NCH + i) * S
            if i < 11:
                # Z for early chunks isn't needed until B2, and the DVE is
                # mostly idle during B1 — reduce E there (into a scratch dest,
                # NOT in-place, so B2's E reads see no extra writer) and spare
                # the ACT queue the 286ns accumulator read.
                nc.scalar.activation(
                    E[:, eb : eb + S],
                    sct[:],
                    mybir.ActivationFunctionType.Exp,
                    scale=SCALE,
                )
                nc.vector.tensor_scalar(
                    zscr[:],
                    E[:, eb : eb + S],
                    1.0,
                    None,
                    mybir.AluOpType.mult,
                    op1=mybir.AluOpType.add,
                    accum_out=stats[:, sb : sb + 1],
                )
            else:
                nc.scalar.activation(
                    E[:, eb : eb + S],
                    sct[:],
                    mybir.ActivationFunctionType.Exp,
                    scale=SCALE,
                    accum_out=stats[:, sb : sb + 1],
                )
            vb = (b * NCH + i) * D
            nc.vector.reciprocal(stats[:, sb + 1 : sb + 2], stats[:, sb : sb + 1])
            # V-scaling on the B1-idle GPSIMD engine: the DVE is saturated by
            # the normalizer reductions, gpsimd has nothing after the V loads
            nc.gpsimd.tensor_scalar_mul(
                Vs[:, vb : vb + D], V[:, vb : vb + D], stats[:, sb + 1 : sb + 2]
            )
    # ---------------- phase B2: dense AV accumulation -----------------------
    # Open the accumulator: each bank gets a full-128-partition zeroing matmul
    # (zero weights) that writes 0 everywhere and sets has_written for the
    # whole bank on every execution, so the partition-sliced AV matmuls can
    # all accumulate with start=False regardless of how the HW scopes the
    # first_mm bank-clear across partitions.
    pot = ps.tile([128, S], FP32, tag="ps", name="pot")
    zmm = []
    for j in range(4):
        zmm.append(
            nc.tensor.matmul(
                pot[:, j * 512 : (j + 1) * 512],
                lhsT=zw[:],
                rhs=QT[:, 0:512],
                start=True,
                stop=False,
                skip_group_check=True,
            )
        )
    o_view = O_all[:].rearrange("p (m b d) -> p m b d", m=NCH, b=BPC, d=D)

    def emit_av(i, j):
        for b in range(BPC):
            # O^T[(b,d), q] += Vs_i^T @ E_i ; b0 -> PE cols 0:63,
            # b1 -> cols 64:127 (concurrent via col tiling)
            vb = (b * NCH + i) * D
            eb = (b * NCH + i) * S
            mm = nc.tensor.matmul(
                pot[b * 64 : (b + 1) * 64, j * 512 : (j + 1) * 512],
                lhsT=Vs[:, vb : vb + D],
                rhs=E[:, eb + j * 512 : eb + (j + 1) * 512],
                start=False,
                stop=(i == NCH - 1 and b == BPC - 1),
                skip_group_check=True,
            )
            if i == 0:
                add_dep_helper(
                    mm.ins,
                    zmm[j].ins,
                    sync=False,
                    reason="AV accumulation after bank-opening zero matmul",
                )

    # dense i-outer accumulation; each region is unpacked (ACT engine) right
    # after its closing matmul so pot's psum slot frees as early as possible —
    # then the 16 out-transposes run with BOTH slots available (2-deep)
    # instead of serializing against the pot-pinned pool.
    for i in range(NCH - 1):
        for j in range(4):
            emit_av(i, j)
    for j in range(4):
        emit_av(NCH - 1, j)
        nc.scalar.copy(
            OT[:, j * 512 : (j + 1) * 512], pot[:, j * 512 : (j + 1) * 512]
        )
    for j in range(4):
        for m in range(4 * j, 4 * j + 4):
            ptc = ps.tile([128, 128], FP32, tag="ps", name=f"ptc_{m}")
            nc.tensor.transpose(ptc[:], OT[:, m * 128 : (m + 1) * 128], ident[:])
            if m % 2 == 0:
                nc.vector.tensor_copy(O_all[:, m * 128 : (m + 1) * 128], ptc[:])
            else:
                nc.scalar.copy(O_all[:, m * 128 : (m + 1) * 128], ptc[:])
        for b in range(BPC):
            nc.sync.dma_start(
                o[b, 4 * j * 128 : (4 * j + 4) * 128, :].rearrange(
                    "(m p) d -> p m d", p=128
                ),
                o_view[:, 4 * j : 4 * j + 4, b, :],
            )


_CACHE: dict = {}


def build_program():
    if "nc" in _CACHE:
        return _CACHE["nc"]
    nc = bacc.Bacc("TRN2", target_bir_lowering=False, debug=False)
    q = nc.dram_tensor("q", [BPC, S, D], FP32, kind="ExternalInput").ap()
    k = nc.dram_tensor("k", [BPC, S, D], FP32, kind="ExternalInput").ap()
    v = nc.dram_tensor("v", [BPC, S, D], FP32, kind="ExternalInput").ap()
    o = nc.dram_tensor("o", [BPC, S, D], FP32, kind="ExternalOutput").ap()
    with tile.TileContext(nc) as tc:
        with ExitStack() as ctx:
            emit_kernel(ctx, tc, q, k, v, o)
    nc.compile()
    _CACHE["nc"] = nc
    return nc


def make_in_maps(q, k, v):
    q = np.ascontiguousarray(q, dtype=np.float32)
    k = np.ascontiguousarray(k, dtype=np.float32)
    v = np.ascontiguousarray(v, dtype=np.float32)
    assert q.shape == (B_FULL, S, D), q.shape
    return [
        {
            "q": np.ascontiguousarray(q[c * BPC : (c + 1) * BPC]),
            "k": np.ascontiguousarray(k[c * BPC : (c + 1) * BPC]),
            "v": np.ascontiguousarray(v[c * BPC : (c + 1) * BPC]),
        }
        for c in range(N_CORES)
    ]


def kernel(q, k, v, _trace=False):
    nc = build_program()
    in_maps = make_in_maps(q, k, v)
    res = bass_utils.run_bass_kernel_spmd(
        nc, in_maps, core_ids=list(range(N_CORES)), trace=_trace
    )
    out = np.concatenate([r["o"] for r in res.results], axis=0)
    if _trace:
        return out, res
    return out
